# revision 40
# baseline (speedup 1.0000x reference)
"""Two-layer GCN (PyG GCNConv x2, relu between) on 8 trn2 NeuronCores.

Strategy (dst-node partitioned, all on-device math):
  - Nodes are sharded across 8 cores by destination row (12500/core),
    each core's rows split in two padded halves so the table AllGathers
    can be issued per half and overlap with compute.
  - Layer tables (dinv * (x@W1), then (dinv*relu(.))@W2) are computed
    shard-wise on-device, AllGather'ed (2 half-collectives per layer)
    into a replicated DRAM table of 256B rows, and per-edge messages are
    fetched with GPSIMD dma_gather.
  - Segment-sum per 128-dst block is a PE matmul with a one-hot selection
    matrix (S) built on DVE via is_equal against an iota row. S is the
    STATIONARY operand (128 cols -> fast weight load) and the gathered
    messages stream as rhs (N=64/40), so PSUM is node-major [dst, ch].
  - Chunk padding is per (superblock, group) cell with shared per-block
    64-multiple slot caps; chunks may straddle one block boundary. The
    second segment's slots encode dstloc+128 and select via a second
    "hi" S matrix (iota 128..255), so every matmul uses full-K operands
    (PE base partitions cannot be offset on HW).
  - Self-loop and bias live in PSUM seeds (eye-weight matmul + rank-1
    sqrt(deg) x bias outer product); the per-block epilogue is a single
    scalar-engine activation with per-partition dinv scale, keeping the
    DVE queue free for S builds (no head-of-line blocking).

The Bass program is identical on all cores (SPMD); per-(cell, block)
slot caps are the max over cores, with padding slots pointing at a zero
table row.
"""

import math
import sys

sys.path.insert(0, "/opt/trn_rl_repo")

import numpy as np


# ---------------------------------------------------------------------------
# configuration
# ---------------------------------------------------------------------------
class Cfg:
    CORES = 8
    N = 100000
    IN_C = 128
    HID = 64
    OUT_C = 40
    NPC = 12500  # nodes per core
    NPC_PAD = 12544  # = 98 * 128, two padded halves of 6272 = 49 * 128
    BLK = 128
    SBB = 4  # dst blocks per superblock (PSUM-tile granularity)
    # int16 unsigned-use reach: the gather ucode treats indices as unsigned
    # in the address math, so only [0, 32767] is usable per window.
    GROUP_ROWS = 32768
    SG = 8  # chunks per S-build op
    CAP_QUANT = 64  # per-block slot-cap quantum
    MAXCH = 8  # max chunks per dma_gather call
    NQ = 4  # SWDGE queues used round-robin
    PSUM_BUFS = 4
    GP_BUFS = 12
    SP_BUFS = 10

    @property
    def HALF(self):
        return self.NPC // 2

    @property
    def HALF_PAD(self):
        return self.NPC_PAD // 2

    @property
    def HBLK(self):
        return self.HALF_PAD // self.BLK  # blocks per half

    @property
    def NBLK(self):
        return self.NPC_PAD // self.BLK

    @property
    def NSB(self):
        return math.ceil(self.NBLK / self.SBB)

    @property
    def TAB(self):
        return self.NPC_PAD * self.CORES

    @property
    def HTAB(self):
        return self.HALF_PAD * self.CORES  # table rows per half

    @property
    def NGRP(self):
        return math.ceil(self.TAB / self.GROUP_ROWS)


# ---------------------------------------------------------------------------
# host-side prep: shard edges, build shared static schedule + per-core arrays
# ---------------------------------------------------------------------------
def _node_map(cfg, node):
    """global node id -> (core, padded local row r_p, global table row srow).

    Table layout is [half][core][half-rows] so each half is one AllGather.
    """
    c = node // cfg.NPC
    r = node - c * cfg.NPC
    h = r // cfg.HALF
    r_h = r - h * cfg.HALF
    r_p = h * cfg.HALF_PAD + r_h
    srow = h * cfg.HTAB + c * cfg.HALF_PAD + r_h
    return c, r_p, srow


def _prepare(cfg, edge_index):
    import ml_dtypes

    src = np.asarray(edge_index[0], dtype=np.int64)
    dst = np.asarray(edge_index[1], dtype=np.int64)
    # deg includes the self-loop; the loop edge itself is applied as a PSUM
    # seed (adds dinv[d]*table_row[d]), not gathered.
    deg = (np.bincount(dst, minlength=cfg.N) + 1.0).astype(np.float32)

    # zero table row per src-group window (per-half core pad rows are zero).
    pad_ranges = []
    for h in range(2):
        for c in range(cfg.CORES):
            base = h * cfg.HTAB + c * cfg.HALF_PAD
            pad_ranges.append((base + cfg.HALF, base + cfg.HALF_PAD))
    zrow = []
    for g in range(cfg.NGRP):
        lo = g * cfg.GROUP_ROWS
        hi = min((g + 1) * cfg.GROUP_ROWS, cfg.TAB)
        r = None
        for p0, p1 in pad_ranges:
            a, b = max(p0, lo), min(p1, hi)
            if a < b:
                r = a
                break
        assert r is not None, f"no zero row available in src-group {g}"
        zrow.append(r)

    owner, dl_all, _ = _node_map(cfg, dst)
    _, _, srow_all = _node_map(cfg, src)
    grp_all = srow_all // cfg.GROUP_ROWS
    blk_all = dl_all // cfg.BLK
    sb_all = blk_all // cfg.SBB

    # per-core edge arrays sorted by (group, sb, block, srow)
    per_core = []
    counts = np.zeros((cfg.CORES, cfg.NSB, cfg.NGRP, cfg.SBB), dtype=np.int64)
    for c in range(cfg.CORES):
        m = owner == c
        srow, dl, grp, blk, sb = (
            srow_all[m], dl_all[m], grp_all[m], blk_all[m], sb_all[m],
        )
        key = ((grp * cfg.NSB + sb) * cfg.NBLK + blk) * (cfg.TAB + 1) + srow
        order = np.argsort(key, kind="stable")
        per_core.append((srow[order], dl[order]))
        np.add.at(counts[c], (sb, grp, blk - sb * cfg.SBB), 1)

    # shared schedule: per (sb, g) cell, per-block slot caps (max over cores,
    # 64-multiples), cell padded to a chunk multiple (extra to last block).
    blockmax = counts.max(axis=0)  # [NSB, NGRP, SBB]
    caps = np.zeros_like(blockmax)
    cell_chunks = np.zeros((cfg.NSB, cfg.NGRP), dtype=np.int64)
    q = cfg.CAP_QUANT
    for s in range(cfg.NSB):
        nvalid = min(cfg.SBB, cfg.NBLK - s * cfg.SBB)
        for g in range(cfg.NGRP):
            bm = (blockmax[s, g] + q - 1) // q * q
            tot = int(bm.sum())
            nch = (tot + cfg.BLK - 1) // cfg.BLK
            bm[nvalid - 1] += nch * cfg.BLK - tot
            caps[s, g] = bm
            cell_chunks[s, g] = nch

    # chunk sequence: group-major, sb-minor (so each group's chunk columns
    # are contiguous and calls can straddle sb boundaries within a group).
    chunk_cell = []  # (g, sb) per chunk
    cell_first_chunk = {}
    for g in range(cfg.NGRP):
        for s in range(cfg.NSB):
            cell_first_chunk[(s, g)] = len(chunk_cell)
            chunk_cell.extend([(g, s)] * int(cell_chunks[s, g]))
    nchunk = len(chunk_cell)
    nslot = nchunk * cfg.BLK

    # matmul segments per chunk: list of (abs_block, k0, k1) slot ranges.
    # With 64-multiple caps a chunk has at most 2 segments.
    segments = [[] for _ in range(nchunk)]
    for s in range(cfg.NSB):
        for g in range(cfg.NGRP):
            j0 = cell_first_chunk[(s, g)]
            pos = 0  # slot position within the cell
            for brel in range(cfg.SBB):
                cap = int(caps[s, g, brel])
                b_abs = s * cfg.SBB + brel
                while cap > 0:
                    j = j0 + pos // cfg.BLK
                    k0 = pos % cfg.BLK
                    take = min(cap, cfg.BLK - k0)
                    segments[j].append((b_abs, k0, k0 + take))
                    pos += take
                    cap -= take
    assert all(len(sg) <= 2 for sg in segments)

    # gather calls: per group, chop the chunk stream into MAXCH pieces;
    # issue order by (sb of first chunk, g).
    calls = []
    for g in range(cfg.NGRP):
        lo = cell_first_chunk[(0, g)]
        hi = lo + int(cell_chunks[:, g].sum())
        j = lo
        while j < hi:
            nch = min(cfg.MAXCH, hi - j)
            calls.append((chunk_cell[j][1], g, j, nch))  # (sb_first, g, j0, nch)
            j += nch
    calls.sort(key=lambda t: (t[0], t[1], t[2]))

    # per-core slot arrays following the shared schedule
    idx_maps = []
    dstloc_maps = []
    for c in range(cfg.CORES):
        srow, dl = per_core[c]
        idx_arr = np.empty(nslot, dtype=np.int64)
        dloc_arr = np.zeros(nslot, dtype=np.int64)
        ei = 0
        for g in range(cfg.NGRP):
            for s in range(cfg.NSB):
                pos = cell_first_chunk[(s, g)] * cfg.BLK
                for brel in range(cfg.SBB):
                    n = int(counts[c, s, g, brel])
                    cap = int(caps[s, g, brel])
                    idx_arr[pos : pos + n] = srow[ei : ei + n] - g * cfg.GROUP_ROWS
                    dloc_arr[pos : pos + n] = dl[ei : ei + n] % cfg.BLK
                    idx_arr[pos + n : pos + cap] = zrow[g] - g * cfg.GROUP_ROWS
                    # pad dstloc stays 0 (gathers a zero row -> adds nothing)
                    pos += cap
                    ei += n
        assert ei == len(srow)
        assert idx_arr.min() >= 0 and idx_arr.max() < cfg.GROUP_ROWS
        # second-segment slots select via the "hi" S matrix (iota 128..255)
        for j in range(nchunk):
            for si, (_, k0, k1) in enumerate(segments[j]):
                if si > 0:
                    dloc_arr[j * cfg.BLK + k0 : j * cfg.BLK + k1] += 128
        idx_maps.append(np.tile(idx_arr.astype(np.int16).reshape(-1, 16).T, (8, 1)))
        dstloc_maps.append(
            np.ascontiguousarray(
                dloc_arr.astype(ml_dtypes.bfloat16).reshape(nchunk, cfg.BLK).T
            )
        )

    return {
        "deg": deg,
        "chunk_cell": chunk_cell,
        "segments": segments,
        "calls": calls,
        "nchunk": nchunk,
        "nslot": nslot,
        "idx_maps": idx_maps,
        "dstloc_maps": dstloc_maps,
    }


# ---------------------------------------------------------------------------
# device program
# ---------------------------------------------------------------------------
def _build(cfg, prep, debug=False):
    import concourse.bacc as bacc
    import concourse.mybir as mybir
    import concourse.tile as tile
    from concourse import library_config
    import ml_dtypes

    fp32 = mybir.dt.float32
    bf16 = mybir.dt.bfloat16
    AF = mybir.ActivationFunctionType
    ALU = mybir.AluOpType

    nchunk = prep["nchunk"]
    segments = prep["segments"]
    calls = prep["calls"]
    chunk_cell = prep["chunk_cell"]

    # per-sb total segment counts (for epilogue emission points)
    sb_nseg = [0] * cfg.NSB
    for j in range(nchunk):
        for b_abs, _, _ in segments[j]:
            sb_nseg[b_abs // cfg.SBB] += 1
    # sbs whose blocks are all in half 0 (their epilogues gate shard-half-0)
    h0_sbs = [s for s in range(cfg.NSB) if (s + 1) * cfg.SBB <= cfg.HBLK]
    h0_gate = set(
        s for s in range(cfg.NSB) if s * cfg.SBB < cfg.HBLK
    )  # sbs containing any half-0 block

    nc = bacc.Bacc(
        "TRN2", target_bir_lowering=False, debug=debug, num_swdge_queues=4
    )

    TW = 128  # table row = 128 bf16 = 256B (dma_gather granularity floor)

    xT_in = nc.dram_tensor("xT", [cfg.IN_C, cfg.NPC_PAD], bf16, kind="ExternalInput")
    W1_in = nc.dram_tensor("W1", [cfg.IN_C, cfg.HID], bf16, kind="ExternalInput")
    W2p_in = nc.dram_tensor("W2p", [cfg.HID, cfg.OUT_C], bf16, kind="ExternalInput")
    b1_in = nc.dram_tensor("b1r", [1, cfg.HID], bf16, kind="ExternalInput")
    b2_in = nc.dram_tensor("b2r", [1, cfg.OUT_C], bf16, kind="ExternalInput")
    sdeg_in = nc.dram_tensor("sdeg", [1, cfg.NPC_PAD], bf16, kind="ExternalInput")
    dinvnw_in = nc.dram_tensor(
        "dinv_nw", [cfg.BLK, cfg.NBLK], fp32, kind="ExternalInput"
    )
    dinv2nw_in = nc.dram_tensor(
        "dinv2_nw", [cfg.BLK, cfg.NBLK], fp32, kind="ExternalInput"
    )
    idx_in = nc.dram_tensor(
        "idxs", [128, (nchunk * cfg.BLK) // 16], mybir.dt.int16, kind="ExternalInput"
    )
    dstloc_in = nc.dram_tensor("dstloc", [cfg.BLK, nchunk], bf16, kind="ExternalInput")
    out_nm = nc.dram_tensor(
        "out_nm", [cfg.NPC_PAD, cfg.OUT_C], fp32, kind="ExternalOutput"
    )
    shard1 = nc.dram_tensor("shard1", [cfg.NPC_PAD, TW], bf16)
    shard2 = nc.dram_tensor("shard2", [cfg.NPC_PAD, TW], bf16)
    table1 = nc.dram_tensor("table1", [cfg.TAB, TW], bf16, addr_space="Shared")
    table2 = nc.dram_tensor("table2", [cfg.TAB, TW], bf16, addr_space="Shared")

    iota_lo_c = nc.inline_tensor(
        np.tile(np.arange(cfg.BLK, dtype=np.float32), (128, cfg.SG))
        .reshape(128, cfg.SG * cfg.BLK)
        .astype(ml_dtypes.bfloat16),
        name="iota_lo",
    )
    iota_hi_c = nc.inline_tensor(
        np.tile(np.arange(128, 128 + cfg.BLK, dtype=np.float32), (128, cfg.SG))
        .reshape(128, cfg.SG * cfg.BLK)
        .astype(ml_dtypes.bfloat16),
        name="iota_hi",
    )
    eye_c = nc.inline_tensor(np.eye(cfg.BLK, dtype=ml_dtypes.bfloat16), name="eye128")

    replica = [list(range(cfg.CORES))]

    def half_allgather(shard, table, h):
        nc.gpsimd.collective_compute(
            "AllGather",
            mybir.AluOpType.bypass,
            replica_groups=replica,
            ins=[shard[h * cfg.HALF_PAD : (h + 1) * cfg.HALF_PAD, :]],
            outs=[table[h * cfg.HTAB : (h + 1) * cfg.HTAB, :]],
        )

    with tile.TileContext(nc) as tc:
        with (
            tc.tile_pool(name="cst", bufs=1) as cst,
            tc.tile_pool(name="gp", bufs=cfg.GP_BUFS) as gp,
            tc.tile_pool(name="sp", bufs=cfg.SP_BUFS) as sp,
            tc.tile_pool(name="ev", bufs=4) as ev,
            tc.tile_pool(name="ot", bufs=3) as otp,
            tc.tile_pool(name="stg", bufs=1) as stg,
        ):
            nc.gpsimd.load_library(library_config.mlp)

            # ---- constants ----
            W1t = cst.tile([cfg.IN_C, cfg.HID], bf16)
            nc.sync.dma_start(W1t[:], W1_in[:])
            W2t = cst.tile([cfg.HID, cfg.OUT_C], bf16)
            nc.sync.dma_start(W2t[:], W2p_in[:])
            b1t = cst.tile([1, cfg.HID], bf16)
            nc.sync.dma_start(b1t[:], b1_in[:])
            b2t = cst.tile([1, cfg.OUT_C], bf16)
            nc.sync.dma_start(b2t[:], b2_in[:])
            sdegt = cst.tile([1, cfg.NPC_PAD], bf16)
            nc.sync.dma_start(sdegt[:], sdeg_in[:])
            iota_lo = cst.tile([128, cfg.SG * cfg.BLK], bf16)
            nc.sync.dma_start(iota_lo[:], iota_lo_c[:])
            iota_hi = cst.tile([128, cfg.SG * cfg.BLK], bf16)
            nc.sync.dma_start(iota_hi[:], iota_hi_c[:])
            eye = cst.tile([cfg.BLK, cfg.BLK], bf16)
            nc.sync.dma_start(eye[:], eye_c[:])
            dinvnw = cst.tile([cfg.BLK, cfg.NBLK], fp32)
            nc.sync.dma_start(dinvnw[:], dinvnw_in[:])
            dinv2nw = cst.tile([cfg.BLK, cfg.NBLK], fp32)
            nc.sync.dma_start(dinv2nw[:], dinv2nw_in[:])
            idxt = cst.tile([128, (nchunk * cfg.BLK) // 16], mybir.dt.int16)
            nc.sync.dma_start(idxt[:], idx_in[:])
            dstloct = cst.tile([cfg.BLK, nchunk], bf16)
            nc.sync.dma_start(dstloct[:], dstloc_in[:])

            # staging for both layer tables (node-major), kept resident:
            # rows double as the self-loop contribution (PSUM seed).
            # Full 256B row width; unused cols zeroed once so the
            # AllGather'ed tables are garbage-free.
            stag1 = stg.tile([cfg.BLK, cfg.NBLK, TW], bf16)
            stag2 = stg.tile([cfg.BLK, cfg.NBLK, TW], bf16)
            nc.vector.memset(stag1[:], 0.0)
            nc.vector.memset(stag2[:], 0.0)
            shard1v = shard1.rearrange("(b p) d -> p b d", p=cfg.BLK)
            shard2v = shard2.rearrange("(b p) d -> p b d", p=cfg.BLK)

            # ---- layer-1 transform: shard1 = dinv * (x @ W1), node-major ----
            with (
                tc.tile_pool(name="xb", bufs=1) as xbp,
                tc.tile_pool(name="psA", bufs=4, space="PSUM") as psA,
            ):
                xbig = xbp.tile([cfg.IN_C, cfg.NPC_PAD], bf16)
                nc.sync.dma_start(xbig[:], xT_in[:])
                for b in range(cfg.NBLK):
                    ps = psA.tile([cfg.BLK, cfg.HID], fp32, tag="pa")
                    nc.tensor.matmul(
                        ps[:],
                        lhsT=xbig[:, b * cfg.BLK : (b + 1) * cfg.BLK],
                        rhs=W1t[:],
                        start=True,
                        stop=True,
                    )
                    nc.vector.tensor_scalar(
                        out=stag1[:, b, : cfg.HID],
                        in0=ps[:],
                        scalar1=dinvnw[:, b : b + 1],
                        scalar2=None,
                        op0=ALU.mult,
                    )
                    if b == cfg.HBLK - 1:
                        nc.sync.dma_start(
                            shard1v[:, : cfg.HBLK, :], stag1[:, : cfg.HBLK, :]
                        )
                        half_allgather(shard1, table1, 0)
                nc.sync.dma_start(
                    shard1v[:, cfg.HBLK :, :], stag1[:, cfg.HBLK :, :]
                )
                half_allgather(shard1, table1, 1)

            # ---- aggregation layer (shared for both layers) ----
            def agg_layer(layer, table, psum_pool, tp_pool, z_pool):
                ch = cfg.HID if layer == 1 else cfg.OUT_C
                stag_self = stag1 if layer == 1 else stag2
                bt = b1t if layer == 1 else b2t
                s_tiles = {}

                def s_for(j):
                    gi = j // cfg.SG
                    if gi not in s_tiles:
                        n = min(cfg.SG, nchunk - gi * cfg.SG)
                        pair = []
                        for nm, iot in (("slo", iota_lo), ("shi", iota_hi)):
                            st = sp.tile(
                                [128, cfg.SG * cfg.BLK], bf16, tag=nm, name=nm
                            )
                            nc.vector.tensor_tensor(
                                out=st[:].rearrange("p (a b) -> p a b", b=cfg.BLK)[
                                    :, :n, :
                                ],
                                in0=iot[:].rearrange("p (a b) -> p a b", b=cfg.BLK)[
                                    :, :n, :
                                ],
                                in1=dstloct[
                                    :, gi * cfg.SG : gi * cfg.SG + n
                                ].to_broadcast([128, n, cfg.BLK]),
                                op=ALU.is_equal,
                            )
                            pair.append(st)
                        s_tiles[gi] = pair
                    return s_tiles[gi], j % cfg.SG

                psum_tiles = {}
                segs_left = list(sb_nseg)
                done_sbs = set()
                shard2a_state = [0]  # 0=pending, >0=countdown, -1=emitted

                def seed(s, pst):
                    # PSUM pre-load: self-loop row (eye-weight matmul; the
                    # first matmul's start marks the whole bank pending-zero)
                    # and the bias as a rank-1 outer product sqrt(deg) x b,
                    # so out = dinv*(sum + self + b/dinv) folds bias in.
                    blo = s * cfg.SBB
                    bhi = min(blo + cfg.SBB, cfg.NBLK)
                    for b in range(blo, bhi):
                        rel = b - blo
                        nc.tensor.matmul(
                            pst[:, rel * ch : rel * ch + ch],
                            lhsT=eye[:],
                            rhs=stag_self[:, b, :ch],
                            start=(b == blo),
                            stop=False,
                        )
                        nc.tensor.matmul(
                            pst[:, rel * ch : rel * ch + ch],
                            lhsT=sdegt[:, b * cfg.BLK : (b + 1) * cfg.BLK],
                            rhs=bt[:],
                            start=False,
                            stop=False,
                        )

                def epilogue(s):
                    blo = s * cfg.SBB
                    bhi = min(blo + cfg.SBB, cfg.NBLK)
                    pst = psum_tiles.pop(s)
                    if layer == 1:
                        for b in range(blo, bhi):
                            rel = b - blo
                            # y = dinv*relu(dinv*acc + b1) = relu(dinv^2*acc')
                            y = ev.tile([cfg.BLK, cfg.HID], bf16, tag="y")
                            nc.scalar.activation(
                                y[:],
                                pst[:, rel * ch : rel * ch + ch],
                                AF.Relu,
                                scale=dinv2nw[:, b : b + 1],
                            )
                            # table2 row: z = y @ W2  (via PE transpose)
                            tp = tp_pool.tile([cfg.HID, cfg.BLK], bf16, tag="tp")
                            nc.tensor.transpose(tp[:], y[:], eye[:])
                            yT = ev.tile([cfg.HID, cfg.BLK], bf16, tag="yT")
                            # y >= 0 so Relu == Copy; reusing Relu avoids the
                            # ~1.3us activation-table swap per function change
                            nc.scalar.activation(yT[:], tp[:], AF.Relu)
                            z = z_pool.tile([cfg.BLK, cfg.OUT_C], fp32, tag="z")
                            nc.tensor.matmul(
                                z[:], lhsT=yT[:], rhs=W2t[:], start=True, stop=True
                            )
                            nc.vector.tensor_copy(stag2[:, b, : cfg.OUT_C], z[:])
                    else:
                        ot = otp.tile([cfg.BLK, cfg.SBB, cfg.OUT_C], fp32, tag="o")
                        for b in range(blo, bhi):
                            rel = b - blo
                            nc.scalar.activation(
                                ot[:, rel, :],
                                pst[:, rel * ch : rel * ch + ch],
                                AF.Copy,
                                scale=dinvnw[:, b : b + 1],
                            )
                        nc.sync.dma_start(
                            out_nm.rearrange("(b p) c -> p b c", p=cfg.BLK)[
                                :, blo:bhi, :
                            ],
                            ot[:, : bhi - blo, :],
                        )
                    done_sbs.add(s)
                    if layer == 1:
                        # ship this superblock's table-2 rows right away so
                        # the half collectives see ready inputs when issued
                        nc.sync.dma_start(
                            shard2v[:, blo:bhi, :], stag2[:, blo:bhi, :]
                        )
                        if shard2a_state[0] == 0 and h0_gate <= done_sbs:
                            shard2a_state[0] = 3  # emit a few calls later

                # pull early group-0 calls (table window entirely in half 0)
                # ahead so they overlap the half-1 AllGather latency; cap the
                # pulled superblock range so open PSUM tiles stay in budget
                pull_sbs = 4 if layer == 1 else 6
                pulled = [
                    i
                    for i, c in enumerate(calls)
                    if c[1] == 0 and chunk_cell[c[2] + c[3] - 1][1] < pull_sbs
                ]
                pset = set(pulled)
                order = pulled + [i for i in range(len(calls)) if i not in pset]
                for ci, oi in enumerate(order):
                    sb_first, g, j0, nch = calls[oi]
                    base = g * cfg.GROUP_ROWS
                    rows = min(cfg.GROUP_ROWS, cfg.TAB - base)
                    gt = gp.tile([128, cfg.MAXCH, TW], bf16, tag="g")
                    nc.gpsimd.dma_gather(
                        gt[:, :nch, :],
                        table[base : base + rows, :],
                        idxt[:, (j0 * cfg.BLK) // 16 : ((j0 + nch) * cfg.BLK) // 16],
                        nch * cfg.BLK,
                        nch * cfg.BLK,
                        TW,
                        queue_num=ci % cfg.NQ,
                    )
                    for j in range(j0, j0 + nch):
                        (st_lo, st_hi), k = s_for(j)
                        for si, (b_abs, k0, k1) in enumerate(segments[j]):
                            s = b_abs // cfg.SBB
                            rel = b_abs - s * cfg.SBB
                            if s not in psum_tiles:
                                psum_tiles[s] = psum_pool.tile(
                                    [cfg.BLK, 512], fp32, tag="ps", name=f"ps{layer}"
                                )
                                seed(s, psum_tiles[s])
                            segs_left[s] -= 1
                            st = st_lo if si == 0 else st_hi
                            nc.tensor.matmul(
                                psum_tiles[s][:, rel * ch : rel * ch + ch],
                                lhsT=st[:, k * cfg.BLK : (k + 1) * cfg.BLK],
                                rhs=gt[:, j - j0, :ch],
                                start=False,
                                stop=(segs_left[s] == 0),
                            )
                    for s in [s for s in psum_tiles if segs_left[s] == 0]:
                        epilogue(s)
                    if shard2a_state[0] > 0:
                        # half-0 of table2 is complete; emit its AllGather a
                        # few calls late so the Pool queue never sits in the
                        # collective's input wait while gathers are pending
                        shard2a_state[0] -= 1
                        if shard2a_state[0] == 0:
                            shard2a_state[0] = -1
                            half_allgather(shard2, table2, 0)
                assert not psum_tiles
                if layer == 1 and shard2a_state[0] != -1:
                    half_allgather(shard2, table2, 0)

            # layer 1 aggregation (+ table2 transform fused in epilogue)
            with (
                tc.tile_pool(name="ps1", bufs=cfg.PSUM_BUFS, space="PSUM") as ps1,
                tc.tile_pool(name="ptp", bufs=2, space="PSUM") as ptp,
                tc.tile_pool(name="pz", bufs=2, space="PSUM") as pz,
            ):
                agg_layer(1, table1, ps1, ptp, pz)
                half_allgather(shard2, table2, 1)

            # layer 2 aggregation -> node-major output
            with tc.tile_pool(name="ps2", bufs=6, space="PSUM") as ps2:
                agg_layer(2, table2, ps2, None, None)

    nc.compile()
    return nc


# ---------------------------------------------------------------------------
# public entry point
# ---------------------------------------------------------------------------
def _make_in_maps(cfg, prep, x, W1, b1, W2, b2):
    import ml_dtypes

    deg = prep["deg"]
    # padded local row for each of a core's nodes, in node order
    r = np.arange(cfg.NPC)
    r_p = (r // cfg.HALF) * cfg.HALF_PAD + (r % cfg.HALF)
    in_maps = []
    for c in range(cfg.CORES):
        xs = np.asarray(x[c * cfg.NPC : (c + 1) * cfg.NPC], np.float32)
        xT = np.zeros((cfg.IN_C, cfg.NPC_PAD), np.float32)
        xT[:, r_p] = xs.T
        # pad nodes: dinv = 0 -> pad table rows and outputs are exactly 0
        dinv = np.zeros(cfg.NPC_PAD, np.float32)
        dinv[r_p] = 1.0 / np.sqrt(deg[c * cfg.NPC : (c + 1) * cfg.NPC])
        sdeg = np.zeros(cfg.NPC_PAD, np.float32)
        sdeg[r_p] = np.sqrt(deg[c * cfg.NPC : (c + 1) * cfg.NPC])
        dinv_nw = np.ascontiguousarray(dinv.reshape(cfg.NBLK, cfg.BLK).T)
        dinv2_nw = np.ascontiguousarray((dinv * dinv).reshape(cfg.NBLK, cfg.BLK).T)
        in_maps.append(
            {
                "xT": xT.astype(ml_dtypes.bfloat16),
                "W1": np.asarray(W1, np.float32).astype(ml_dtypes.bfloat16),
                "W2p": np.asarray(W2, np.float32).astype(ml_dtypes.bfloat16),
                "b1r": np.asarray(b1, np.float32).reshape(1, -1).astype(
                    ml_dtypes.bfloat16
                ),
                "b2r": np.asarray(b2, np.float32).reshape(1, -1).astype(
                    ml_dtypes.bfloat16
                ),
                "sdeg": sdeg.reshape(1, -1).astype(ml_dtypes.bfloat16),
                "dinv_nw": dinv_nw,
                "dinv2_nw": dinv2_nw,
                "idxs": prep["idx_maps"][c],
                "dstloc": prep["dstloc_maps"][c],
            }
        )
    return in_maps


def _run(cfg, inputs, mode="hw", trace=False):
    x = np.asarray(inputs["x"], np.float32)
    edge_index = np.asarray(inputs["edge_index"])
    W1 = np.asarray(inputs["W1"], np.float32)
    b1 = np.asarray(inputs["b1"], np.float32)
    W2 = np.asarray(inputs["W2"], np.float32)
    b2 = np.asarray(inputs["b2"], np.float32)

    prep = _prepare(cfg, edge_index)
    nc = _build(cfg, prep, debug=(mode == "sim"))
    in_maps = _make_in_maps(cfg, prep, x, W1, b1, W2, b2)

    info = {}
    if mode == "sim":
        from concourse.bass_interp import MultiCoreSim

        sim = MultiCoreSim(nc, cfg.CORES)
        for c in range(cfg.CORES):
            for k, v in in_maps[c].items():
                sim.cores[c].tensor(k)[:] = v
        sim.simulate()
        outs = [sim.cores[c].tensor("out_nm").copy() for c in range(cfg.CORES)]
    else:
        import concourse.bass_utils as bu

        if trace:
            # avoid the S3 artifact upload in the profile path
            bu.upload_artifacts = lambda d: "(local)"
        r = bu.run_bass_kernel_spmd(
            nc, in_maps, list(range(cfg.CORES)), trace=trace,
            tmpdir=(inputs.get("_tracedir") if trace else None),
        )
        info["exec_time_ns"] = r.exec_time_ns
        info["mean_exec_time_ns"] = r.mean_exec_time_ns
        outs = [r.results[c]["out_nm"] for c in range(cfg.CORES)]

    r = np.arange(cfg.NPC)
    r_p = (r // cfg.HALF) * cfg.HALF_PAD + (r % cfg.HALF)
    out = np.concatenate([o[r_p, :] for o in outs], axis=0)
    return out.astype(np.float32), info


def kernel(**inputs):
    out, _ = _run(Cfg(), inputs, mode="hw")
    return out


# revision 42
# speedup vs baseline: 1.1010x; 1.1010x over previous
"""Two-layer GCN (PyG GCNConv x2, relu between) on 8 trn2 NeuronCores.

Strategy (dst-node partitioned, all on-device math):
  - Nodes are sharded across 8 cores by destination row (12500/core),
    each core's rows split in two padded halves so the table AllGathers
    can be issued per half and overlap with compute.
  - Layer tables (dinv * (x@W1), then (dinv*relu(.))@W2) are computed
    shard-wise on-device, AllGather'ed (2 half-collectives per layer)
    into a replicated DRAM table of 256B rows, and per-edge messages are
    fetched with GPSIMD dma_gather.
  - Segment-sum per 128-dst block is a PE matmul with a one-hot selection
    matrix (S) built on DVE via is_equal against an iota row. S is the
    STATIONARY operand (128 cols -> fast weight load) and the gathered
    messages stream as rhs (N=64/40), so PSUM is node-major [dst, ch].
  - Chunk padding is per (superblock, group) cell with shared per-block
    64-multiple slot caps; chunks may straddle one block boundary. The
    second segment's slots encode dstloc+128 and select via a second
    "hi" S matrix (iota 128..255), so every matmul uses full-K operands
    (PE base partitions cannot be offset on HW).
  - Self-loop and bias live in PSUM seeds (eye-weight matmul + rank-1
    sqrt(deg) x bias outer product); the per-block epilogue is a single
    scalar-engine activation with per-partition dinv scale, keeping the
    DVE queue free for S builds (no head-of-line blocking).

The Bass program is identical on all cores (SPMD); per-(cell, block)
slot caps are the max over cores, with padding slots pointing at a zero
table row.
"""

import math
import sys

sys.path.insert(0, "/opt/trn_rl_repo")

import numpy as np


# ---------------------------------------------------------------------------
# configuration
# ---------------------------------------------------------------------------
class Cfg:
    CORES = 8
    N = 100000
    IN_C = 128
    HID = 64
    OUT_C = 40
    NPC = 12500  # nodes per core
    NPC_PAD = 12544  # = 98 * 128, two padded halves of 6272 = 49 * 128
    BLK = 128
    SBB = 4  # dst blocks per superblock (PSUM-tile granularity)
    # int16 unsigned-use reach: the gather ucode treats indices as unsigned
    # in the address math, so only [0, 32767] is usable per window.
    GROUP_ROWS = 32768
    SG = 8  # chunks per S-build op
    CAP_QUANT = 64  # per-block slot-cap quantum
    MAXCH = 8  # max chunks per dma_gather call
    NQ = 4  # SWDGE queues used round-robin
    PSUM_BUFS = 4
    GP_BUFS = 12
    SP_BUFS = 8

    @property
    def HALF(self):
        return self.NPC // 2

    @property
    def HALF_PAD(self):
        return self.NPC_PAD // 2

    @property
    def HBLK(self):
        return self.HALF_PAD // self.BLK  # blocks per half

    @property
    def NBLK(self):
        return self.NPC_PAD // self.BLK

    @property
    def NSB(self):
        return math.ceil(self.NBLK / self.SBB)

    @property
    def TAB(self):
        return self.NPC_PAD * self.CORES

    @property
    def HTAB(self):
        return self.HALF_PAD * self.CORES  # table rows per half

    @property
    def NGRP(self):
        return math.ceil(self.TAB / self.GROUP_ROWS)


# ---------------------------------------------------------------------------
# host-side prep: shard edges, build shared static schedule + per-core arrays
# ---------------------------------------------------------------------------
def _node_map(cfg, node):
    """global node id -> (core, padded local row r_p, global table row srow).

    Table layout is [half][core][half-rows] so each half is one AllGather.
    """
    c = node // cfg.NPC
    r = node - c * cfg.NPC
    h = r // cfg.HALF
    r_h = r - h * cfg.HALF
    r_p = h * cfg.HALF_PAD + r_h
    srow = h * cfg.HTAB + c * cfg.HALF_PAD + r_h
    return c, r_p, srow


def _prepare(cfg, edge_index):
    import ml_dtypes

    src = np.asarray(edge_index[0], dtype=np.int64)
    dst = np.asarray(edge_index[1], dtype=np.int64)
    # deg includes the self-loop; the loop edge itself is applied as a PSUM
    # seed (adds dinv[d]*table_row[d]), not gathered.
    deg = (np.bincount(dst, minlength=cfg.N) + 1.0).astype(np.float32)

    # zero table row per src-group window (per-half core pad rows are zero).
    pad_ranges = []
    for h in range(2):
        for c in range(cfg.CORES):
            base = h * cfg.HTAB + c * cfg.HALF_PAD
            pad_ranges.append((base + cfg.HALF, base + cfg.HALF_PAD))
    zrow = []
    for g in range(cfg.NGRP):
        lo = g * cfg.GROUP_ROWS
        hi = min((g + 1) * cfg.GROUP_ROWS, cfg.TAB)
        r = None
        for p0, p1 in pad_ranges:
            a, b = max(p0, lo), min(p1, hi)
            if a < b:
                r = a
                break
        assert r is not None, f"no zero row available in src-group {g}"
        zrow.append(r)

    owner, dl_all, _ = _node_map(cfg, dst)
    _, _, srow_all = _node_map(cfg, src)
    grp_all = srow_all // cfg.GROUP_ROWS
    blk_all = dl_all // cfg.BLK
    sb_all = blk_all // cfg.SBB

    # per-core edge arrays sorted by (group, sb, block, srow)
    per_core = []
    counts = np.zeros((cfg.CORES, cfg.NSB, cfg.NGRP, cfg.SBB), dtype=np.int64)
    for c in range(cfg.CORES):
        m = owner == c
        srow, dl, grp, blk, sb = (
            srow_all[m], dl_all[m], grp_all[m], blk_all[m], sb_all[m],
        )
        key = ((grp * cfg.NSB + sb) * cfg.NBLK + blk) * (cfg.TAB + 1) + srow
        order = np.argsort(key, kind="stable")
        per_core.append((srow[order], dl[order]))
        np.add.at(counts[c], (sb, grp, blk - sb * cfg.SBB), 1)

    # shared schedule: per (sb, g) cell, per-block slot caps (max over cores,
    # 64-multiples), cell padded to a chunk multiple (extra to last block).
    blockmax = counts.max(axis=0)  # [NSB, NGRP, SBB]
    caps = np.zeros_like(blockmax)
    cell_chunks = np.zeros((cfg.NSB, cfg.NGRP), dtype=np.int64)
    q = cfg.CAP_QUANT
    for s in range(cfg.NSB):
        nvalid = min(cfg.SBB, cfg.NBLK - s * cfg.SBB)
        for g in range(cfg.NGRP):
            bm = (blockmax[s, g] + q - 1) // q * q
            tot = int(bm.sum())
            nch = (tot + cfg.BLK - 1) // cfg.BLK
            bm[nvalid - 1] += nch * cfg.BLK - tot
            caps[s, g] = bm
            cell_chunks[s, g] = nch

    # chunk sequence: group-major, sb-minor (so each group's chunk columns
    # are contiguous and calls can straddle sb boundaries within a group).
    chunk_cell = []  # (g, sb) per chunk
    cell_first_chunk = {}
    for g in range(cfg.NGRP):
        for s in range(cfg.NSB):
            cell_first_chunk[(s, g)] = len(chunk_cell)
            chunk_cell.extend([(g, s)] * int(cell_chunks[s, g]))
    nchunk = len(chunk_cell)
    nslot = nchunk * cfg.BLK

    # matmul segments per chunk: list of (abs_block, k0, k1) slot ranges.
    # With 64-multiple caps a chunk has at most 2 segments.
    segments = [[] for _ in range(nchunk)]
    for s in range(cfg.NSB):
        for g in range(cfg.NGRP):
            j0 = cell_first_chunk[(s, g)]
            pos = 0  # slot position within the cell
            for brel in range(cfg.SBB):
                cap = int(caps[s, g, brel])
                b_abs = s * cfg.SBB + brel
                while cap > 0:
                    j = j0 + pos // cfg.BLK
                    k0 = pos % cfg.BLK
                    take = min(cap, cfg.BLK - k0)
                    segments[j].append((b_abs, k0, k0 + take))
                    pos += take
                    cap -= take
    assert all(len(sg) <= 2 for sg in segments)

    # gather calls: per group, chop the chunk stream into MAXCH pieces;
    # issue order by (sb of first chunk, g).
    calls = []
    for g in range(cfg.NGRP):
        lo = cell_first_chunk[(0, g)]
        hi = lo + int(cell_chunks[:, g].sum())
        j = lo
        while j < hi:
            nch = min(cfg.MAXCH, hi - j)
            calls.append((chunk_cell[j][1], g, j, nch))  # (sb_first, g, j0, nch)
            j += nch
    calls.sort(key=lambda t: (t[0], t[1], t[2]))

    # per-core slot arrays following the shared schedule
    idx_maps = []
    dstloc_maps = []
    for c in range(cfg.CORES):
        srow, dl = per_core[c]
        idx_arr = np.empty(nslot, dtype=np.int64)
        dloc_arr = np.zeros(nslot, dtype=np.int64)
        ei = 0
        for g in range(cfg.NGRP):
            for s in range(cfg.NSB):
                pos = cell_first_chunk[(s, g)] * cfg.BLK
                for brel in range(cfg.SBB):
                    n = int(counts[c, s, g, brel])
                    cap = int(caps[s, g, brel])
                    idx_arr[pos : pos + n] = srow[ei : ei + n] - g * cfg.GROUP_ROWS
                    dloc_arr[pos : pos + n] = dl[ei : ei + n] % cfg.BLK
                    idx_arr[pos + n : pos + cap] = zrow[g] - g * cfg.GROUP_ROWS
                    # pad dstloc stays 0 (gathers a zero row -> adds nothing)
                    pos += cap
                    ei += n
        assert ei == len(srow)
        assert idx_arr.min() >= 0 and idx_arr.max() < cfg.GROUP_ROWS
        # second-segment slots select via the "hi" S matrix (iota 128..255)
        for j in range(nchunk):
            for si, (_, k0, k1) in enumerate(segments[j]):
                if si > 0:
                    dloc_arr[j * cfg.BLK + k0 : j * cfg.BLK + k1] += 128
        idx_maps.append(np.tile(idx_arr.astype(np.int16).reshape(-1, 16).T, (8, 1)))
        dstloc_maps.append(
            np.ascontiguousarray(
                dloc_arr.astype(ml_dtypes.bfloat16).reshape(nchunk, cfg.BLK).T
            )
        )

    return {
        "deg": deg,
        "chunk_cell": chunk_cell,
        "segments": segments,
        "calls": calls,
        "nchunk": nchunk,
        "nslot": nslot,
        "idx_maps": idx_maps,
        "dstloc_maps": dstloc_maps,
    }


# ---------------------------------------------------------------------------
# device program
# ---------------------------------------------------------------------------
def _build(cfg, prep, debug=False):
    import concourse.bacc as bacc
    import concourse.mybir as mybir
    import concourse.tile as tile
    from concourse import library_config
    import ml_dtypes

    fp32 = mybir.dt.float32
    bf16 = mybir.dt.bfloat16
    AF = mybir.ActivationFunctionType
    ALU = mybir.AluOpType

    nchunk = prep["nchunk"]
    segments = prep["segments"]
    calls = prep["calls"]
    chunk_cell = prep["chunk_cell"]

    # per-sb total segment counts (for epilogue emission points)
    sb_nseg = [0] * cfg.NSB
    for j in range(nchunk):
        for b_abs, _, _ in segments[j]:
            sb_nseg[b_abs // cfg.SBB] += 1
    # sbs whose blocks are all in half 0 (their epilogues gate shard-half-0)
    h0_sbs = [s for s in range(cfg.NSB) if (s + 1) * cfg.SBB <= cfg.HBLK]
    h0_gate = set(
        s for s in range(cfg.NSB) if s * cfg.SBB < cfg.HBLK
    )  # sbs containing any half-0 block

    nc = bacc.Bacc(
        "TRN2", target_bir_lowering=False, debug=debug, num_swdge_queues=4
    )

    TW = 128  # table row = 128 bf16 = 256B (dma_gather granularity floor)

    xT_in = nc.dram_tensor("xT", [cfg.IN_C, cfg.NPC_PAD], bf16, kind="ExternalInput")
    W1_in = nc.dram_tensor("W1", [cfg.IN_C, cfg.HID], bf16, kind="ExternalInput")
    W2p_in = nc.dram_tensor("W2p", [cfg.HID, cfg.OUT_C], bf16, kind="ExternalInput")
    b1_in = nc.dram_tensor("b1r", [1, cfg.HID], bf16, kind="ExternalInput")
    b2_in = nc.dram_tensor("b2r", [1, cfg.OUT_C], bf16, kind="ExternalInput")
    sdeg_in = nc.dram_tensor("sdeg", [1, cfg.NPC_PAD], bf16, kind="ExternalInput")
    dinvnw_in = nc.dram_tensor(
        "dinv_nw", [cfg.BLK, cfg.NBLK], fp32, kind="ExternalInput"
    )
    dinv2nw_in = nc.dram_tensor(
        "dinv2_nw", [cfg.BLK, cfg.NBLK], fp32, kind="ExternalInput"
    )
    idx_in = nc.dram_tensor(
        "idxs", [128, (nchunk * cfg.BLK) // 16], mybir.dt.int16, kind="ExternalInput"
    )
    dstloc_in = nc.dram_tensor("dstloc", [cfg.BLK, nchunk], bf16, kind="ExternalInput")
    out_nm = nc.dram_tensor(
        "out_nm", [cfg.NPC_PAD, cfg.OUT_C], fp32, kind="ExternalOutput"
    )
    shard1 = nc.dram_tensor("shard1", [cfg.NPC_PAD, TW], bf16)
    shard2 = nc.dram_tensor("shard2", [cfg.NPC_PAD, TW], bf16)
    table1 = nc.dram_tensor("table1", [cfg.TAB, TW], bf16, addr_space="Shared")
    table2 = nc.dram_tensor("table2", [cfg.TAB, TW], bf16, addr_space="Shared")

    iota_lo_c = nc.inline_tensor(
        np.tile(np.arange(cfg.BLK, dtype=np.float32), (128, cfg.SG))
        .reshape(128, cfg.SG * cfg.BLK)
        .astype(ml_dtypes.bfloat16),
        name="iota_lo",
    )
    iota_hi_c = nc.inline_tensor(
        np.tile(np.arange(128, 128 + cfg.BLK, dtype=np.float32), (128, cfg.SG))
        .reshape(128, cfg.SG * cfg.BLK)
        .astype(ml_dtypes.bfloat16),
        name="iota_hi",
    )
    eye_c = nc.inline_tensor(np.eye(cfg.BLK, dtype=ml_dtypes.bfloat16), name="eye128")

    replica = [list(range(cfg.CORES))]

    def half_allgather(shard, table, h):
        nc.gpsimd.collective_compute(
            "AllGather",
            mybir.AluOpType.bypass,
            replica_groups=replica,
            ins=[shard[h * cfg.HALF_PAD : (h + 1) * cfg.HALF_PAD, :]],
            outs=[table[h * cfg.HTAB : (h + 1) * cfg.HTAB, :]],
        )

    with tile.TileContext(nc) as tc:
        with (
            tc.tile_pool(name="cst", bufs=1) as cst,
            tc.tile_pool(name="gp", bufs=cfg.GP_BUFS) as gp,
            tc.tile_pool(name="sp", bufs=cfg.SP_BUFS) as sp,
            tc.tile_pool(name="ev", bufs=4) as ev,
            tc.tile_pool(name="ot", bufs=3) as otp,
            tc.tile_pool(name="stg", bufs=1) as stg,
        ):
            nc.gpsimd.load_library(library_config.mlp)

            # ---- constants ----
            W1t = cst.tile([cfg.IN_C, cfg.HID], bf16)
            nc.sync.dma_start(W1t[:], W1_in[:])
            W2t = cst.tile([cfg.HID, cfg.OUT_C], bf16)
            nc.sync.dma_start(W2t[:], W2p_in[:])
            b1t = cst.tile([1, cfg.HID], bf16)
            nc.sync.dma_start(b1t[:], b1_in[:])
            b2t = cst.tile([1, cfg.OUT_C], bf16)
            nc.sync.dma_start(b2t[:], b2_in[:])
            sdegt = cst.tile([1, cfg.NPC_PAD], bf16)
            nc.sync.dma_start(sdegt[:], sdeg_in[:])
            iota_lo = cst.tile([128, cfg.SG * cfg.BLK], bf16)
            nc.sync.dma_start(iota_lo[:], iota_lo_c[:])
            iota_hi = cst.tile([128, cfg.SG * cfg.BLK], bf16)
            nc.sync.dma_start(iota_hi[:], iota_hi_c[:])
            eye = cst.tile([cfg.BLK, cfg.BLK], bf16)
            nc.sync.dma_start(eye[:], eye_c[:])
            dinvnw = cst.tile([cfg.BLK, cfg.NBLK], fp32)
            nc.sync.dma_start(dinvnw[:], dinvnw_in[:])
            dinv2nw = cst.tile([cfg.BLK, cfg.NBLK], fp32)
            nc.sync.dma_start(dinv2nw[:], dinv2nw_in[:])
            idxt = cst.tile([128, (nchunk * cfg.BLK) // 16], mybir.dt.int16)
            nc.sync.dma_start(idxt[:], idx_in[:])
            dstloct = cst.tile([cfg.BLK, nchunk], bf16)
            nc.sync.dma_start(dstloct[:], dstloc_in[:])

            # staging for both layer tables (node-major), kept resident:
            # rows double as the self-loop contribution (PSUM seed).
            # Full 256B row width; unused cols zeroed once so the
            # AllGather'ed tables are garbage-free.
            stag1 = stg.tile([cfg.BLK, cfg.NBLK, TW], bf16)
            stag2 = stg.tile([cfg.BLK, cfg.NBLK, TW], bf16)
            nc.vector.memset(stag1[:], 0.0)
            nc.vector.memset(stag2[:], 0.0)
            shard1v = shard1.rearrange("(b p) d -> p b d", p=cfg.BLK)
            shard2v = shard2.rearrange("(b p) d -> p b d", p=cfg.BLK)

            # ---- layer-1 transform: shard1 = dinv * (x @ W1), node-major ----
            with (
                tc.tile_pool(name="xb", bufs=1) as xbp,
                tc.tile_pool(name="psA", bufs=4, space="PSUM") as psA,
            ):
                xbig = xbp.tile([cfg.IN_C, cfg.NPC_PAD], bf16)
                nc.sync.dma_start(xbig[:], xT_in[:])
                for b in range(cfg.NBLK):
                    ps = psA.tile([cfg.BLK, cfg.HID], fp32, tag="pa")
                    nc.tensor.matmul(
                        ps[:],
                        lhsT=xbig[:, b * cfg.BLK : (b + 1) * cfg.BLK],
                        rhs=W1t[:],
                        start=True,
                        stop=True,
                    )
                    nc.vector.tensor_scalar(
                        out=stag1[:, b, : cfg.HID],
                        in0=ps[:],
                        scalar1=dinvnw[:, b : b + 1],
                        scalar2=None,
                        op0=ALU.mult,
                    )
                    if b == cfg.HBLK - 1:
                        nc.sync.dma_start(
                            shard1v[:, : cfg.HBLK, :], stag1[:, : cfg.HBLK, :]
                        )
                        half_allgather(shard1, table1, 0)
                nc.sync.dma_start(
                    shard1v[:, cfg.HBLK :, :], stag1[:, cfg.HBLK :, :]
                )
                half_allgather(shard1, table1, 1)

            # ---- aggregation layer (shared for both layers) ----
            def agg_layer(layer, table, psum_pool, tp_pool, z_pool):
                ch = cfg.HID if layer == 1 else cfg.OUT_C
                stag_self = stag1 if layer == 1 else stag2
                bt = b1t if layer == 1 else b2t
                s_tiles = {}

                def s_for(j):
                    gi = j // cfg.SG
                    if gi not in s_tiles:
                        n = min(cfg.SG, nchunk - gi * cfg.SG)
                        pair = []
                        for nm, iot in (("slo", iota_lo), ("shi", iota_hi)):
                            st = sp.tile(
                                [128, cfg.SG * cfg.BLK], bf16, tag=nm, name=nm
                            )
                            nc.vector.tensor_tensor(
                                out=st[:].rearrange("p (a b) -> p a b", b=cfg.BLK)[
                                    :, :n, :
                                ],
                                in0=iot[:].rearrange("p (a b) -> p a b", b=cfg.BLK)[
                                    :, :n, :
                                ],
                                in1=dstloct[
                                    :, gi * cfg.SG : gi * cfg.SG + n
                                ].to_broadcast([128, n, cfg.BLK]),
                                op=ALU.is_equal,
                            )
                            pair.append(st)
                        s_tiles[gi] = pair
                    return s_tiles[gi], j % cfg.SG

                psum_tiles = {}
                segs_left = list(sb_nseg)
                done_sbs = set()
                shard2a_state = [0]  # 0=pending, >0=countdown, -1=emitted

                def seed(s, pst):
                    # PSUM pre-load: self-loop row (eye-weight matmul; the
                    # first matmul's start marks the whole bank pending-zero)
                    # and the bias as a rank-1 outer product sqrt(deg) x b,
                    # so out = dinv*(sum + self + b/dinv) folds bias in.
                    blo = s * cfg.SBB
                    bhi = min(blo + cfg.SBB, cfg.NBLK)
                    for b in range(blo, bhi):
                        rel = b - blo
                        nc.tensor.matmul(
                            pst[:, rel * ch : rel * ch + ch],
                            lhsT=eye[:],
                            rhs=stag_self[:, b, :ch],
                            start=(b == blo),
                            stop=False,
                        )
                        nc.tensor.matmul(
                            pst[:, rel * ch : rel * ch + ch],
                            lhsT=sdegt[:, b * cfg.BLK : (b + 1) * cfg.BLK],
                            rhs=bt[:],
                            start=False,
                            stop=False,
                        )

                def epilogue(s):
                    blo = s * cfg.SBB
                    bhi = min(blo + cfg.SBB, cfg.NBLK)
                    pst = psum_tiles.pop(s)
                    if layer == 1:
                        for b in range(blo, bhi):
                            rel = b - blo
                            # y = dinv*relu(dinv*acc + b1) = relu(dinv^2*acc')
                            y = ev.tile([cfg.BLK, cfg.HID], bf16, tag="y")
                            nc.scalar.activation(
                                y[:],
                                pst[:, rel * ch : rel * ch + ch],
                                AF.Relu,
                                scale=dinv2nw[:, b : b + 1],
                            )
                            # table2 row: z = y @ W2  (via PE transpose)
                            tp = tp_pool.tile([cfg.HID, cfg.BLK], bf16, tag="tp")
                            nc.tensor.transpose(tp[:], y[:], eye[:])
                            yT = ev.tile([cfg.HID, cfg.BLK], bf16, tag="yT")
                            # y >= 0 so Relu == Copy; reusing Relu avoids the
                            # ~1.3us activation-table swap per function change
                            nc.scalar.activation(yT[:], tp[:], AF.Relu)
                            z = z_pool.tile([cfg.BLK, cfg.OUT_C], fp32, tag="z")
                            nc.tensor.matmul(
                                z[:], lhsT=yT[:], rhs=W2t[:], start=True, stop=True
                            )
                            nc.vector.tensor_copy(stag2[:, b, : cfg.OUT_C], z[:])
                    else:
                        ot = otp.tile([cfg.BLK, cfg.SBB, cfg.OUT_C], fp32, tag="o")
                        for b in range(blo, bhi):
                            rel = b - blo
                            nc.scalar.activation(
                                ot[:, rel, :],
                                pst[:, rel * ch : rel * ch + ch],
                                AF.Copy,
                                scale=dinvnw[:, b : b + 1],
                            )
                        nc.sync.dma_start(
                            out_nm.rearrange("(b p) c -> p b c", p=cfg.BLK)[
                                :, blo:bhi, :
                            ],
                            ot[:, : bhi - blo, :],
                        )
                    done_sbs.add(s)
                    if layer == 1:
                        # ship this superblock's table-2 rows right away so
                        # the half collectives see ready inputs when issued
                        nc.sync.dma_start(
                            shard2v[:, blo:bhi, :], stag2[:, blo:bhi, :]
                        )
                        if shard2a_state[0] == 0 and h0_gate <= done_sbs:
                            shard2a_state[0] = 3  # emit a few calls later

                for ci, (sb_first, g, j0, nch) in enumerate(calls):
                    base = g * cfg.GROUP_ROWS
                    rows = min(cfg.GROUP_ROWS, cfg.TAB - base)
                    gt = gp.tile([128, cfg.MAXCH, TW], bf16, tag="g")
                    nc.gpsimd.dma_gather(
                        gt[:, :nch, :],
                        table[base : base + rows, :],
                        idxt[:, (j0 * cfg.BLK) // 16 : ((j0 + nch) * cfg.BLK) // 16],
                        nch * cfg.BLK,
                        nch * cfg.BLK,
                        TW,
                        queue_num=ci % cfg.NQ,
                    )
                    for j in range(j0, j0 + nch):
                        (st_lo, st_hi), k = s_for(j)
                        for si, (b_abs, k0, k1) in enumerate(segments[j]):
                            s = b_abs // cfg.SBB
                            rel = b_abs - s * cfg.SBB
                            if s not in psum_tiles:
                                psum_tiles[s] = psum_pool.tile(
                                    [cfg.BLK, 512], fp32, tag="ps", name=f"ps{layer}"
                                )
                                seed(s, psum_tiles[s])
                            segs_left[s] -= 1
                            st = st_lo if si == 0 else st_hi
                            nc.tensor.matmul(
                                psum_tiles[s][:, rel * ch : rel * ch + ch],
                                lhsT=st[:, k * cfg.BLK : (k + 1) * cfg.BLK],
                                rhs=gt[:, j - j0, :ch],
                                start=False,
                                stop=(segs_left[s] == 0),
                            )
                    for s in [s for s in psum_tiles if segs_left[s] == 0]:
                        epilogue(s)
                    if shard2a_state[0] > 0:
                        # half-0 of table2 is complete; emit its AllGather a
                        # few calls late so the Pool queue never sits in the
                        # collective's input wait while gathers are pending
                        shard2a_state[0] -= 1
                        if shard2a_state[0] == 0:
                            shard2a_state[0] = -1
                            half_allgather(shard2, table2, 0)
                assert not psum_tiles
                if layer == 1 and shard2a_state[0] != -1:
                    half_allgather(shard2, table2, 0)

            # layer 1 aggregation (+ table2 transform fused in epilogue)
            with (
                tc.tile_pool(name="ps1", bufs=cfg.PSUM_BUFS, space="PSUM") as ps1,
                tc.tile_pool(name="ptp", bufs=2, space="PSUM") as ptp,
                tc.tile_pool(name="pz", bufs=2, space="PSUM") as pz,
            ):
                agg_layer(1, table1, ps1, ptp, pz)
                half_allgather(shard2, table2, 1)

            # layer 2 aggregation -> node-major output
            with tc.tile_pool(name="ps2", bufs=6, space="PSUM") as ps2:
                agg_layer(2, table2, ps2, None, None)

    nc.compile()
    return nc


# ---------------------------------------------------------------------------
# public entry point
# ---------------------------------------------------------------------------
def _make_in_maps(cfg, prep, x, W1, b1, W2, b2):
    import ml_dtypes

    deg = prep["deg"]
    # padded local row for each of a core's nodes, in node order
    r = np.arange(cfg.NPC)
    r_p = (r // cfg.HALF) * cfg.HALF_PAD + (r % cfg.HALF)
    in_maps = []
    for c in range(cfg.CORES):
        xs = np.asarray(x[c * cfg.NPC : (c + 1) * cfg.NPC], np.float32)
        xT = np.zeros((cfg.IN_C, cfg.NPC_PAD), np.float32)
        xT[:, r_p] = xs.T
        # pad nodes: dinv = 0 -> pad table rows and outputs are exactly 0
        dinv = np.zeros(cfg.NPC_PAD, np.float32)
        dinv[r_p] = 1.0 / np.sqrt(deg[c * cfg.NPC : (c + 1) * cfg.NPC])
        sdeg = np.zeros(cfg.NPC_PAD, np.float32)
        sdeg[r_p] = np.sqrt(deg[c * cfg.NPC : (c + 1) * cfg.NPC])
        dinv_nw = np.ascontiguousarray(dinv.reshape(cfg.NBLK, cfg.BLK).T)
        dinv2_nw = np.ascontiguousarray((dinv * dinv).reshape(cfg.NBLK, cfg.BLK).T)
        in_maps.append(
            {
                "xT": xT.astype(ml_dtypes.bfloat16),
                "W1": np.asarray(W1, np.float32).astype(ml_dtypes.bfloat16),
                "W2p": np.asarray(W2, np.float32).astype(ml_dtypes.bfloat16),
                "b1r": np.asarray(b1, np.float32).reshape(1, -1).astype(
                    ml_dtypes.bfloat16
                ),
                "b2r": np.asarray(b2, np.float32).reshape(1, -1).astype(
                    ml_dtypes.bfloat16
                ),
                "sdeg": sdeg.reshape(1, -1).astype(ml_dtypes.bfloat16),
                "dinv_nw": dinv_nw,
                "dinv2_nw": dinv2_nw,
                "idxs": prep["idx_maps"][c],
                "dstloc": prep["dstloc_maps"][c],
            }
        )
    return in_maps


def _run(cfg, inputs, mode="hw", trace=False):
    x = np.asarray(inputs["x"], np.float32)
    edge_index = np.asarray(inputs["edge_index"])
    W1 = np.asarray(inputs["W1"], np.float32)
    b1 = np.asarray(inputs["b1"], np.float32)
    W2 = np.asarray(inputs["W2"], np.float32)
    b2 = np.asarray(inputs["b2"], np.float32)

    prep = _prepare(cfg, edge_index)
    nc = _build(cfg, prep, debug=(mode == "sim"))
    in_maps = _make_in_maps(cfg, prep, x, W1, b1, W2, b2)

    info = {}
    if mode == "sim":
        from concourse.bass_interp import MultiCoreSim

        sim = MultiCoreSim(nc, cfg.CORES)
        for c in range(cfg.CORES):
            for k, v in in_maps[c].items():
                sim.cores[c].tensor(k)[:] = v
        sim.simulate()
        outs = [sim.cores[c].tensor("out_nm").copy() for c in range(cfg.CORES)]
    else:
        import concourse.bass_utils as bu

        if trace:
            # avoid the S3 artifact upload in the profile path
            bu.upload_artifacts = lambda d: "(local)"
        r = bu.run_bass_kernel_spmd(
            nc, in_maps, list(range(cfg.CORES)), trace=trace,
            tmpdir=(inputs.get("_tracedir") if trace else None),
        )
        info["exec_time_ns"] = r.exec_time_ns
        info["mean_exec_time_ns"] = r.mean_exec_time_ns
        outs = [r.results[c]["out_nm"] for c in range(cfg.CORES)]

    r = np.arange(cfg.NPC)
    r_p = (r // cfg.HALF) * cfg.HALF_PAD + (r % cfg.HALF)
    out = np.concatenate([o[r_p, :] for o in outs], axis=0)
    return out.astype(np.float32), info


def kernel(**inputs):
    out, _ = _run(Cfg(), inputs, mode="hw")
    return out
